# revision 8
# baseline (speedup 1.0000x reference)
"""Trainium2 Bass kernel for mixed-head attention (CIM attention).

Reference computation (per batch element b):
    qkv  = x @ w_qkv.T                                  [N, 3C]
    q,k,v split into H=4 heads of HD=128
    S_h  = (q_h @ k_h.T) * SCALE                        [N, N] per head
    S'_i = sum_h M[i,h] * S_h        (CIM head mix)
    A_i  = softmax(S'_i, axis=-1)
    O_i  = A_i @ v_i
    out  = concat_i(O_i) @ w_proj.T + b_proj

Distribution: data-parallel over B=8, one batch element per NeuronCore.
No collectives needed; host shards/gathers.

Single-core algorithm (all matmuls bf16 with fp32 PSUM accumulation):
  - Host ships x^T, w_qkv^T, w_proj^T pre-transposed and pre-cast to bf16,
    so the contraction dim is always on SBUF partitions. No device transposes.
  - The CIM mix is folded into Q: Qhat_i[(h,d), n] = M[i,h]*SCALE*Q_h[d, n],
    produced by 4 scaled PSUM->SBUF copies of each Q projection tile
    (ACT activation Copy with per-partition scale).  Score matmul then
    contracts over all 512 (h,d) pairs: S'_i^T[m, n] = Qhat_i^T-contract-K.
  - Scores live in [m_part, n_free] ("S^T") layout so exp is elementwise and
    attn@v needs no transpose: O_i^T[d, n] = sum_m V[m, d] * expS_i^T[m, n].
  - Softmax normalization is deferred: rowsum_i[n] = sum_m expS_i^T[m, n]
    computed with a ones[128,128] stationary matmul (broadcasts the sum to
    all 128 partitions for free), then O_i^T * (1/rowsum) on DVE.
  - proj: out[n, c] = sum_{(i,d)} Onorm_i^T[(i,d), n] * w_proj^T[(i,d), c],
    bias added as a K=1 ones-outer-product accumulation matmul.
"""

import os
import sys

for _p in ("/opt/trn_rl_repo",):
    if os.path.isdir(_p) and _p not in sys.path:
        sys.path.insert(0, _p)

import numpy as np
import ml_dtypes

import concourse.bass as bass
import concourse.tile as tile
from concourse import bacc, mybir
from concourse.bass_utils import run_bass_kernel_spmd

B, N, C, H = 8, 1024, 512, 4
HD = C // H          # 128
SCALE = HD ** -0.5
NCORES = 8
P = 128              # partitions
NCH = N // 512       # 512-wide free-dim chunks per N
NB = N // P          # 128-row blocks per N
CB = C // P          # 128-row blocks per C

BF16 = mybir.dt.bfloat16
FP32 = mybir.dt.float32
AF = mybir.ActivationFunctionType


def _mix_matrix_np(w_main: np.ndarray, w_rest: np.ndarray) -> np.ndarray:
    rows = np.repeat(np.arange(H), H - 1)
    cols = np.array([[j for j in range(H) if j != i] for i in range(H)]).ravel()
    M = np.zeros((H, H), dtype=np.float64)
    M[rows, cols] = w_rest.astype(np.float64).ravel()
    M += np.diag(w_main.astype(np.float64))
    return M


def build_graph():
    nc = bacc.Bacc(
        "TRN2",
        target_bir_lowering=False,
        debug=False,
        num_devices=NCORES,
    )

    # packed "SBUF image" layouts: [128, CB*W], block cb at free-offset cb*W
    xT = nc.dram_tensor("xT", [P, CB * N], BF16, kind="ExternalInput").ap()
    wqk = nc.dram_tensor("wqk", [P, CB * 2 * C], BF16, kind="ExternalInput").ap()
    wv = nc.dram_tensor("wv", [P, CB * C], BF16, kind="ExternalInput").ap()
    wpTp = nc.dram_tensor("wpTp", [P, CB * C], BF16, kind="ExternalInput").ap()
    bprow = nc.dram_tensor("bprow", [1, C], BF16, kind="ExternalInput").ap()
    qscales = nc.dram_tensor("qscales", [P, H * H], FP32, kind="ExternalInput").ap()
    out = nc.dram_tensor("out", [N, C], FP32, kind="ExternalOutput").ap()

    with tile.TileContext(nc) as tc:
        with (
            tc.tile_pool(name="const", bufs=1) as cpool,
            tc.tile_pool(name="wts", bufs=1) as wpool,
            tc.tile_pool(name="qkv", bufs=1) as qkvpool,
            tc.tile_pool(name="es", bufs=12) as espool,
            tc.tile_pool(name="onorm", bufs=1) as opool,
            tc.tile_pool(name="outsb", bufs=3) as outpool,
            tc.tile_pool(name="ps2", bufs=2, space="PSUM") as ps2pool,
            tc.tile_pool(name="psmm", bufs=3, space="PSUM") as psmm,
        ):
            # ---- packed DMA loads (one issue per tensor; ~600ns/issue) ----
            # Each SBUF tile packs the CB=4 partition-blocks side by side:
            # block cb lives at free-offset cb*width.
            wqkp = wpool.tile([P, CB * 2 * C], BF16, tag="wqkp", name="wqkp")
            nc.sync.dma_start(wqkp[:], wqk[:, :])
            xtp = wpool.tile([P, CB * N], BF16, tag="xtp", name="xtp")
            nc.scalar.dma_start(xtp[:], xT[:, :])
            wvp = wpool.tile([P, CB * C], BF16, tag="wvp", name="wvp")
            nc.sync.dma_start(wvp[:], wv[:, :])
            wpp = wpool.tile([P, CB * C], BF16, tag="wpp", name="wpp")
            nc.scalar.dma_start(wpp[:], wpTp[:, :])
            wqk_sb = [wqkp[:, cb * 2 * C:(cb + 1) * 2 * C] for cb in range(CB)]
            xt_sb = [xtp[:, cb * N:(cb + 1) * N] for cb in range(CB)]
            wv_sb = [wvp[:, cb * C:(cb + 1) * C] for cb in range(CB)]
            wp_sb = [wpp[:, cb * C:(cb + 1) * C] for cb in range(CB)]

            # constants
            ones_m = cpool.tile([P, P], BF16, tag="ones_m")
            nc.gpsimd.memset(ones_m[:], 1.0)
            ones_1 = cpool.tile([1, P], BF16, tag="ones_1")
            nc.gpsimd.memset(ones_1[:], 1.0)
            qsc = cpool.tile([P, H * H], FP32, tag="qsc")
            nc.sync.dma_start(qsc[:], qscales[:, :])
            bpr = cpool.tile([1, C], BF16, tag="bpr")
            nc.sync.dma_start(bpr[:], bprow[:, :])

            # ---- QKV projections ----
            # qhat[i][h]: [128(d), N] bf16 ; kt[h]: [128(d), N] ; v[mb]: [128(m), C]
            qhat = [[qkvpool.tile([P, N], BF16, tag=f"qhat{i}_{h}",
                                  name=f"qhat{i}_{h}")
                     for h in range(H)] for i in range(H)]
            kt = [qkvpool.tile([P, N], BF16, tag=f"kt{h}", name=f"kt{h}")
                  for h in range(H)]
            v_sb = [qkvpool.tile([P, C], BF16, tag=f"v{mb}", name=f"v{mb}")
                    for mb in range(NB)]

            # Q and K: 2-bank PSUM tile [128, 1024] per j-block, then one
            # 1024-wide epilogue copy (split across ACT and DVE).
            for jb in range(2 * H):          # 0-3: Q heads, 4-7: K heads
                ps2 = ps2pool.tile([P, N], FP32, tag="mm2", name=f"qk_ps{jb}")
                for ch in range(NCH):
                    for cb in range(CB):
                        nc.tensor.matmul(
                            ps2[:, ch * 512:(ch + 1) * 512],
                            wqk_sb[cb][:, jb * P:(jb + 1) * P],
                            xt_sb[cb][:, ch * 512:(ch + 1) * 512],
                            start=(cb == 0), stop=(cb == CB - 1),
                        )
                if jb < H:
                    h = jb
                    # one PSUM->SBUF cast on ACT, then 4 cheap bf16 4x-mode
                    # scaled copies on DVE (CIM mix scales folded into qhat)
                    qb = qkvpool.tile([P, N], BF16, tag=f"qb{h}", name=f"qb{h}")
                    nc.scalar.copy(qb[:], ps2[:])
                    for i in range(H):
                        sc = qsc[:, i * H + h:i * H + h + 1]
                        nc.vector.tensor_scalar_mul(qhat[i][h][:], qb[:], sc)
                else:
                    h = jb - H
                    nc.scalar.copy(kt[h][:], ps2[:])

            # V: out[m_block, c] = sum_cb xT[cb][:, mblk].T @ wvT[cb]
            for mb in range(NB):
                ps = psmm.tile([P, 512], FP32, tag="mm", name=f"v_ps{mb}")
                for cb in range(CB):
                    nc.tensor.matmul(
                        ps[:],
                        xt_sb[cb][:, mb * P:(mb + 1) * P],
                        wv_sb[cb][:],
                        start=(cb == 0), stop=(cb == CB - 1),
                    )
                nc.vector.tensor_copy(v_sb[mb][:], ps[:])

            # ---- chunk-outer head loop: scores -> exp -> rowsum/attnv ----
            onorm = [opool.tile([P, N], BF16, tag=f"onorm{i}", name=f"onorm{i}")
                     for i in range(H)]

            for ch in range(NCH):
                nsl = slice(ch * 512, (ch + 1) * 512)
                for i in range(H):
                    es = [espool.tile([P, 512], BF16, tag="es",
                                      name=f"es{ch}_{i}_{mb}")
                          for mb in range(NB)]
                    ps_rso = ps2pool.tile([P, N], FP32, tag="mm2",
                                          name=f"rso{ch}_{i}")
                    ps_rs = ps_rso[:, 0:512]
                    ps_o = ps_rso[:, 512:1024]
                    for mb in range(NB):
                        ps = psmm.tile([P, 512], FP32, tag="mm",
                                       name=f"s_ps{ch}_{i}_{mb}")
                        for h in range(H):
                            nc.tensor.matmul(
                                ps[:],
                                kt[h][:, mb * P:(mb + 1) * P],
                                qhat[i][h][:, nsl],
                                start=(h == 0), stop=(h == H - 1),
                            )
                        nc.scalar.activation(es[mb][:], ps[:], AF.Exp)
                    # rowsum (ones lhsT broadcasts the sum to all 128
                    # partitions), then attn@v, accumulated over m blocks.
                    # Separate chains so the ones LDWEIGHTS is reused.
                    for mb in range(NB):
                        nc.tensor.matmul(
                            ps_rs, ones_m[:], es[mb][:],
                            start=(mb == 0), stop=(mb == NB - 1),
                        )
                    for mb in range(NB):
                        nc.tensor.matmul(
                            ps_o, v_sb[mb][:, i * P:(i + 1) * P], es[mb][:],
                            start=(mb == 0), stop=(mb == NB - 1),
                        )
                    rec = outpool.tile([P, 512], FP32, tag="rec",
                                       name=f"rec{ch}_{i}")
                    nc.vector.reciprocal_approx_fast(rec[:], ps_rs)
                    nc.vector.tensor_mul(onorm[i][:, nsl], ps_o, rec[:])

                # ---- output projection + bias for this chunk's n rows ----
                for nb in range(ch * NB // NCH, (ch + 1) * NB // NCH):
                    ps = psmm.tile([P, 512], FP32, tag="mm", name=f"p_ps{nb}")
                    nc.tensor.matmul(ps[:], ones_1[:], bpr[:],
                                     start=True, stop=False)
                    for ib in range(H):
                        nc.tensor.matmul(
                            ps[:],
                            onorm[ib][:, nb * P:(nb + 1) * P],
                            wp_sb[ib][:],
                            start=False, stop=(ib == H - 1),
                        )
                    osb = outpool.tile([P, 512], FP32, tag="osb",
                                       name=f"osb{nb}")
                    nc.vector.tensor_copy(osb[:], ps[:])
                    nc.sync.dma_start(out[nb * P:(nb + 1) * P, :], osb[:])

    nc.compile()
    return nc


def _pack(a):
    """[C, W] -> [128, CB*W] SBUF image: partition-block cb at offset cb*W."""
    Crows, W = a.shape
    return np.ascontiguousarray(
        a.reshape(Crows // P, P, W).transpose(1, 0, 2).reshape(P, -1)
    )


def make_in_maps(x, w_qkv, w_proj, b_proj, w_main, w_rest):
    M = _mix_matrix_np(np.asarray(w_main), np.asarray(w_rest))
    bf = ml_dtypes.bfloat16
    wqkvT = np.ascontiguousarray(np.asarray(w_qkv, np.float32).T).astype(bf)
    wpT = np.ascontiguousarray(np.asarray(w_proj, np.float32).T).astype(bf)
    bprow = np.asarray(b_proj, np.float32).reshape(1, C).astype(bf)
    qs = np.empty((P, H * H), np.float32)
    for i in range(H):
        for h in range(H):
            qs[:, i * H + h] = np.float32(M[i, h] * SCALE)
    x = np.asarray(x, np.float32)
    wqk_p = _pack(wqkvT[:, 0:2 * C])
    wv_p = _pack(wqkvT[:, 2 * C:3 * C])
    wp_p = _pack(wpT)
    in_maps = []
    for b in range(B):
        in_maps.append({
            "xT": _pack(np.ascontiguousarray(x[b].T).astype(bf)),
            "wqk": wqk_p,
            "wv": wv_p,
            "wpTp": wp_p,
            "bprow": bprow,
            "qscales": qs,
        })
    return in_maps


_NC_CACHE = {}


def get_graph():
    if "nc" not in _NC_CACHE:
        _NC_CACHE["nc"] = build_graph()
    return _NC_CACHE["nc"]


def kernel(x, w_qkv, w_proj, b_proj, w_main, w_rest, _trace=False, _trace_kwargs=None):
    nc = get_graph()
    in_maps = make_in_maps(x, w_qkv, w_proj, b_proj, w_main, w_rest)
    kw = {}
    if _trace:
        kw = {"trace": True}
        if _trace_kwargs:
            kw.update(_trace_kwargs)
    res = run_bass_kernel_spmd(nc, in_maps, core_ids=list(range(NCORES)), **kw)
    outb = np.stack([res.results[i]["out"] for i in range(NCORES)], axis=0)
    if _trace:
        return outb.astype(np.float32), res
    return outb.astype(np.float32)
